# revision 14
# baseline (speedup 1.0000x reference)
"""
BDHAttention (strictly-causal linear attention with interleaved RoPE) on 8
Trainium2 NeuronCores.

Full shapes: Q,K,V [2, 12, 2048, 256] fp32 -> out [2, 12, 2048, 256] fp32.
Sharding: the 24 (batch, head) attention instances are data-parallel, 3 per
core. Each core runs the same NEFF on its own slice.

Per-instance algorithm (T=2048 tokens in 16 chunks of 128, grouped in pairs):
  - RoPE is applied to Q and K in a de-interleaved ("evens then odds") lane
    order. Because every matmul only ever CONTRACTS over the feature axis,
    a consistent permutation of that axis on both sides is a no-op, so the
    de-interleaved order is never undone. The rotation tables are
    pre-permuted on the host and halved (cos[2k] == cos[2k+1]).
  - Intra-group (256 tokens): S~ = (KR QR^T) for the 2x2 chunk block,
    strict-causal mask on the diagonal 128-blocks, then out += S~^T-matmuls
    with V.
  - Inter-group: a running state = sum_{s<group} KR[s]^T V[s] ([256,256],
    fp32 in PSUM); out += QR @ state. State is updated after use.
"""

import math

import numpy as np

P = 128
T = 2048
N = 256
NI = 3  # instances per core
N_CORES = 8
CHUNKS = 16  # T / P
HALF_CH = 8  # chunks per rope/staging half
GROUPS = 8  # groups of 2 chunks
THETA = 2.0 ** 16

_CACHE = {}


def _tables():
    """Half-size rope tables in the de-interleaved lane order, bf16."""
    import ml_dtypes

    j = np.arange(0, N, 2, dtype=np.float32)  # even lanes; q = floor(i/2)*2 = j
    freqs = (
        np.float32(1.0)
        / np.power(np.float32(THETA), (j / np.float32(N)), dtype=np.float32)
        / np.float32(2.0 * math.pi)
    ).astype(np.float32)
    t = np.arange(T, dtype=np.float32)[:, None]
    phases = (t * freqs[None, :]).astype(np.float32)
    ph = np.mod(phases, np.float32(1.0)) * np.float32(2.0 * math.pi)
    cosh = np.cos(ph).astype(np.float32)
    sinh = np.sin(ph).astype(np.float32)
    return cosh.astype(ml_dtypes.bfloat16), sinh.astype(ml_dtypes.bfloat16)


def _build():
    import concourse.bacc as bacc
    import concourse.mybir as mybir
    import concourse.tile as tile
    from concourse.masks import make_identity, make_upper_triangular

    f32 = mybir.dt.float32
    bf16 = mybir.dt.bfloat16

    nc = bacc.Bacc(None, target_bir_lowering=False)
    Q = nc.declare_dram_parameter("Q", [NI, T, N], f32, isOutput=False)
    K = nc.declare_dram_parameter("K", [NI, T, N], f32, isOutput=False)
    V = nc.declare_dram_parameter("V", [NI, T, N], f32, isOutput=False)
    COSH = nc.declare_dram_parameter("COSH", [T, N // 2], bf16, isOutput=False)
    SINH = nc.declare_dram_parameter("SINH", [T, N // 2], bf16, isOutput=False)
    O = nc.declare_dram_parameter("O", [NI, T, N], f32, isOutput=True)

    # chunk-major views: [p, chunk, n]
    q_v = Q.rearrange("i (c p) n -> i p c n", p=P)
    k_v = K.rearrange("i (c p) n -> i p c n", p=P)
    v_v = V.rearrange("i (c p) n -> i p c n", p=P)
    o_v = O.rearrange("i (c p) n -> i p c n", p=P)

    with tile.TileContext(nc) as tc:
        const = tc.alloc_tile_pool(name="const", bufs=1)
        stage = tc.alloc_tile_pool(name="stage", bufs=2)
        dei = tc.alloc_tile_pool(name="dei", bufs=2)
        ab = tc.alloc_tile_pool(name="ab", bufs=2)
        rk = tc.alloc_tile_pool(name="rk", bufs=2)
        tsb = tc.alloc_tile_pool(name="tsb", bufs=2)
        ssb = tc.alloc_tile_pool(name="ssb", bufs=2)
        stateb_p = tc.alloc_tile_pool(name="stateb", bufs=2)
        trans_p = tc.alloc_tile_pool(name="trans", bufs=2, space="PSUM")
        smm_p = tc.alloc_tile_pool(name="smm", bufs=2, space="PSUM")
        state_p = tc.alloc_tile_pool(name="state", bufs=2, space="PSUM")
        outp_p = tc.alloc_tile_pool(name="outp", bufs=2, space="PSUM")

        # constants
        cos_sb = const.tile([P, CHUNKS, 128], bf16)
        sin_sb = const.tile([P, CHUNKS, 128], bf16)
        nc.sync.dma_start(out=cos_sb, in_=COSH.rearrange("(c p) j -> p c j", p=P))
        nc.sync.dma_start(out=sin_sb, in_=SINH.rearrange("(c p) j -> p c j", p=P))
        ident = const.tile([P, P], bf16)
        make_identity(nc, ident)
        # mask[s, t] = 1.0 iff s < t  (strictly upper triangular)
        maskS = const.tile([P, P], bf16)
        make_upper_triangular(nc, maskS, val=1.0, diag=False)

        for inst in range(NI):
            # one PSUM bank per accumulation group (start=True zeroes a whole
            # 2KB zero-region, so groups may never share a live bank)
            state_ps = [
                state_p.tile([P, 256], f32, tag="st", name=f"st{inst}_{h}")
                for h in (0, 1)
            ]
            for half in range(2):
                cs = slice(half * HALF_CH, (half + 1) * HALF_CH)
                qf = stage.tile([P, HALF_CH, N], f32, tag="qf")
                kf = stage.tile([P, HALF_CH, N], f32, tag="kf")
                vf = stage.tile([P, HALF_CH, N], f32, tag="vf")
                nc.sync.dma_start(out=qf, in_=q_v[inst, :, cs, :])
                nc.sync.dma_start(out=kf, in_=k_v[inst, :, cs, :])
                nc.sync.dma_start(out=vf, in_=v_v[inst, :, cs, :])

                vb = rk.tile([P, HALF_CH, N], bf16, tag="vb")
                nc.any.tensor_copy(vb, vf)

                # de-interleave + cast: [..., 0, :] = even lanes, [..., 1, :] = odd
                cos_b = (
                    cos_sb[:, cs, :].unsqueeze(2).broadcast_to([P, HALF_CH, 2, 128])
                )
                sin_b = (
                    sin_sb[:, cs, :].unsqueeze(2).broadcast_to([P, HALF_CH, 2, 128])
                )
                rots = []
                for name, xf in (("q", qf), ("k", kf)):
                    xpair = xf.rearrange("p c (j two) -> p c j two", two=2)
                    xde = dei.tile([P, HALF_CH, 2, 128], bf16, tag=name + "de")
                    nc.any.tensor_copy(xde[:, :, 0, :], xpair[:, :, :, 0])
                    nc.any.tensor_copy(xde[:, :, 1, :], xpair[:, :, :, 1])
                    a_t = ab.tile([P, HALF_CH, 2, 128], bf16, tag="a")
                    b_t = ab.tile([P, HALF_CH, 2, 128], bf16, tag="b")
                    nc.vector.tensor_mul(a_t, xde, cos_b)
                    nc.vector.tensor_mul(b_t, xde, sin_b)
                    xr = rk.tile([P, HALF_CH, 2, 128], bf16, tag=name + "r")
                    # XR_even = A_even - B_odd ; XR_odd = A_odd + B_even
                    nc.vector.tensor_sub(
                        xr[:, :, 0, :], a_t[:, :, 0, :], b_t[:, :, 1, :]
                    )
                    nc.vector.tensor_add(
                        xr[:, :, 1, :], a_t[:, :, 1, :], b_t[:, :, 0, :]
                    )
                    rots.append(xr)
                qr, kr = rots

                for gg in range(4):
                    g = half * 4 + gg  # global group
                    d0, d1 = 2 * gg, 2 * gg + 1  # chunk idx within half
                    c0 = 2 * g  # global chunk of first half of group

                    # --- transposes: [t,n~] -> [n~,t] (bf16, via PE) ---
                    tq = trans_p.tile([P, 512], bf16, tag="tps")
                    tk = trans_p.tile([P, 512], bf16, tag="tps")
                    for pos, d in ((0, d0), (1, d1)):
                        for h in (0, 1):
                            off = h * 256 + pos * 128
                            nc.tensor.transpose(
                                tq[:, off : off + 128], qr[:, d, h, :], ident
                            )
                            nc.tensor.transpose(
                                tk[:, off : off + 128], kr[:, d, h, :], ident
                            )
                    qrt = tsb.tile([P, 512], bf16, tag="qrt")
                    krt = tsb.tile([P, 512], bf16, tag="krt")
                    nc.any.tensor_copy(qrt, tq)
                    nc.any.tensor_copy(krt, tk)

                    # --- S~[s, t] for the 2x2 chunk block of this group ---
                    # rows: s in chunk c0 -> cols 0:256 over t in (c0,c1)
                    #       s in chunk c1 -> cols 256:512 (only t in c1 used)
                    # groups sharing stp's bank must be strictly sequential
                    stp = smm_p.tile([P, 512], f32)
                    for sblk in (0, 1):
                        for h in (0, 1):
                            hh = h * 256
                            nc.tensor.matmul(
                                stp[:, sblk * 256 : sblk * 256 + 256],
                                lhsT=krt[:, hh + sblk * 128 : hh + sblk * 128 + 128],
                                rhs=qrt[:, hh : hh + 256],
                                start=(h == 0),
                                stop=(h == 1),
                            )
                    sts = ssb.tile([P, 512], bf16)
                    nc.vector.tensor_mul(sts[:, 0:128], stp[:, 0:128], maskS)
                    nc.any.tensor_copy(sts[:, 128:256], stp[:, 128:256])
                    nc.vector.tensor_mul(sts[:, 384:512], stp[:, 384:512], maskS)

                    # --- inter-group state snapshot (before this group's update)
                    if g > 0:
                        stateb = stateb_p.tile([P, 512], bf16)
                        nc.any.tensor_copy(stateb[:, 0:256], state_ps[0])
                        nc.any.tensor_copy(stateb[:, 256:512], state_ps[1])

                    # --- outputs for chunks c0, c1 (one bank each) ---
                    op0 = outp_p.tile([P, 256], f32, tag="op")
                    op1 = outp_p.tile([P, 256], f32, tag="op")
                    nc.tensor.matmul(
                        op0,
                        lhsT=sts[:, 0:128],
                        rhs=vb[:, d0, :],
                        start=True,
                        stop=(g == 0),
                    )
                    nc.tensor.matmul(
                        op1,
                        lhsT=sts[:, 128:256],
                        rhs=vb[:, d0, :],
                        start=True,
                        stop=False,
                    )
                    nc.tensor.matmul(
                        op1,
                        lhsT=sts[:, 384:512],
                        rhs=vb[:, d1, :],
                        start=False,
                        stop=(g == 0),
                    )
                    if g > 0:
                        nc.tensor.matmul(
                            op0,
                            lhsT=qrt[:, 0:128],
                            rhs=stateb[:, 0:256],
                            start=False,
                            stop=False,
                        )
                        nc.tensor.matmul(
                            op0,
                            lhsT=qrt[:, 256:384],
                            rhs=stateb[:, 256:512],
                            start=False,
                            stop=True,
                        )
                        nc.tensor.matmul(
                            op1,
                            lhsT=qrt[:, 128:256],
                            rhs=stateb[:, 0:256],
                            start=False,
                            stop=False,
                        )
                        nc.tensor.matmul(
                            op1,
                            lhsT=qrt[:, 384:512],
                            rhs=stateb[:, 256:512],
                            start=False,
                            stop=True,
                        )

                    # --- state update (not needed after last group) ---
                    if g < GROUPS - 1:
                        for h in (0, 1):
                            nc.tensor.matmul(
                                state_ps[h],
                                lhsT=kr[:, d0, h, :],
                                rhs=vb[:, d0, :],
                                start=(g == 0),
                                stop=False,
                            )
                            nc.tensor.matmul(
                                state_ps[h],
                                lhsT=kr[:, d1, h, :],
                                rhs=vb[:, d1, :],
                                start=False,
                                stop=(g == GROUPS - 2),
                            )

                    # --- write out both chunks (PSUM -> SBUF -> DRAM) ---
                    osb = ssb.tile([P, 512], f32, tag="osb")
                    nc.any.tensor_copy(osb[:, 0:256], op0)
                    nc.any.tensor_copy(osb[:, 256:512], op1)
                    nc.sync.dma_start(
                        out=o_v[inst, :, c0 : c0 + 2, :],
                        in_=osb.rearrange("p (c n) -> p c n", c=2),
                    )

        outp_p.release()
        state_p.release()
        smm_p.release()
        trans_p.release()
        stateb_p.release()
        ssb.release()
        tsb.release()
        rk.release()
        ab.release()
        dei.release()
        stage.release()
        const.release()

    nc.compile()
    return nc


def _get_nc():
    if "nc" not in _CACHE:
        _CACHE["nc"] = _build()
        _CACHE["tables"] = _tables()
    return _CACHE["nc"]


def _run(inputs, trace=False):
    from concourse.bass_utils import run_bass_kernel_spmd

    nc = _get_nc()
    cosh, sinh = _CACHE["tables"]

    q = np.ascontiguousarray(np.asarray(inputs["Q"], dtype=np.float32)).reshape(
        24, T, N
    )
    k = np.ascontiguousarray(np.asarray(inputs["K"], dtype=np.float32)).reshape(
        24, T, N
    )
    v = np.ascontiguousarray(np.asarray(inputs["V"], dtype=np.float32)).reshape(
        24, T, N
    )

    in_maps = []
    for c in range(N_CORES):
        s = slice(c * NI, (c + 1) * NI)
        in_maps.append(
            {
                "Q": np.ascontiguousarray(q[s]),
                "K": np.ascontiguousarray(k[s]),
                "V": np.ascontiguousarray(v[s]),
                "COSH": cosh,
                "SINH": sinh,
            }
        )

    res = run_bass_kernel_spmd(nc, in_maps, list(range(N_CORES)), trace=trace)
    out = np.concatenate([res.results[c]["O"] for c in range(N_CORES)], axis=0)
    return out.reshape(2, 12, T, N).astype(np.float32), res


def kernel(**inputs):
    out, _ = _run(inputs, trace=False)
    return out


def bench(iters=30, **inputs):
    """Time steady-state NEFF executions.

    The container has no NTFF profile hook, so measure by queueing `iters`
    async executions of the sharded jitted NEFF (inputs resident on device)
    and dividing the blocked wall time. Per-call JAX dispatch overlaps with
    device execution, so per-iter ~= on-device exec time.
    """
    import jax
    import jax.numpy as jnp
    from jax.sharding import Mesh, PartitionSpec
    from jax.experimental.shard_map import shard_map
    import concourse.mybir as mybir
    from concourse import bass2jax
    import time

    out = kernel(**inputs)  # correctness path (and warms compile caches)

    nc = _get_nc()
    bass2jax.install_neuronx_cc_hook()
    cosh, sinh = _CACHE["tables"]

    q = np.asarray(inputs["Q"], dtype=np.float32).reshape(24, T, N)
    k = np.asarray(inputs["K"], dtype=np.float32).reshape(24, T, N)
    v = np.asarray(inputs["V"], dtype=np.float32).reshape(24, T, N)

    part_name = nc.partition_id_tensor.name if nc.partition_id_tensor else None
    in_names = []
    out_names = []
    out_avals = []
    for alloc in nc.m.functions[0].allocations:
        if not isinstance(alloc, mybir.MemoryLocationSet):
            continue
        name = alloc.memorylocations[0].name
        if alloc.kind == "ExternalInput":
            if name != part_name:
                in_names.append(name)
        elif alloc.kind == "ExternalOutput":
            out_names.append(name)
            out_avals.append(
                jax.core.ShapedArray(
                    tuple(alloc.tensor_shape), mybir.dt.np(alloc.dtype)
                )
            )
    all_names = in_names + out_names
    if part_name is not None:
        all_names.append(part_name)

    host = {
        "Q": q,
        "K": k,
        "V": v,
        "COSH": np.broadcast_to(cosh, (N_CORES,) + cosh.shape).reshape(
            N_CORES * cosh.shape[0], cosh.shape[1]
        ),
        "SINH": np.broadcast_to(sinh, (N_CORES,) + sinh.shape).reshape(
            N_CORES * sinh.shape[0], sinh.shape[1]
        ),
    }

    def _body(*args):
        outs = bass2jax._bass_exec_p.bind(
            *args,
            out_avals=tuple(out_avals),
            in_names=tuple(all_names),
            out_names=tuple(out_names),
            lowering_input_output_aliases=(),
            sim_require_finite=True,
            sim_require_nnan=True,
            nc=nc,
        )
        return tuple(outs)

    devices = jax.devices()[:N_CORES]
    mesh = Mesh(np.asarray(devices), ("core",))
    nin = len(in_names) + len(out_avals) + (1 if part_name is not None else 0)
    fn = jax.jit(
        shard_map(
            _body,
            mesh=mesh,
            in_specs=(PartitionSpec("core"),) * nin,
            out_specs=(PartitionSpec("core"),) * len(out_names),
            check_rep=False,
        ),
        keep_unused=True,
    )
    args = [host[n] for n in in_names] + [
        np.zeros((N_CORES * a.shape[0],) + a.shape[1:], a.dtype) for a in out_avals
    ]
    if part_name is not None:
        args.append(np.arange(N_CORES, dtype=np.uint32).reshape(N_CORES, 1))
    dev_args = [jax.device_put(a) for a in args]
    # warmup (compile + first exec)
    r = fn(*dev_args)
    jax.block_until_ready(r)
    t0 = time.perf_counter()
    for _ in range(iters):
        r = fn(*dev_args)
    jax.block_until_ready(r)
    t1 = time.perf_counter()
    per_iter_ns = (t1 - t0) / iters * 1e9
    # sanity: timed path must agree with the graded path
    timed_out = (
        np.asarray(r[0]).reshape(N_CORES * NI, T, N).reshape(2, 12, T, N)
    )
    assert np.allclose(timed_out, out, atol=1e-5), "timed path diverged"
    return out, per_iter_ns
